# revision 5
# baseline (speedup 1.0000x reference)
"""Trainium2 Bass kernel for nn_DFTQNN_81776177316168.

reference: probs = |U_24 ... U_1 psi|^2 with U_k = expm(-i theta_k G_k),
G_k = (M_k + M_k^H)/2 Hermitian 1024x1024, psi = normalized padded feature.

Strategy (expert-parallel, per sharding hint):
  - 24 gates across 8 cores, 3 per core. Gates ranked by spectral-norm bound
    a_k = |theta_k| * lam_bound; slot j of every core holds ranks j*8..j*8+7,
    so slot squaring count s_j is uniform across cores (SPMD single program).
  - Per gate on device: M = (theta/2^s) G (built by PE-transpose symmetrize),
    V0 = exp(-iM) by degree-11 Taylor in Paterson-Stockmeyer form
    (powers M^2, M^3, then 3 Horner steps with the per-chunk linear term
    fused into the PSUM eviction), then s repeated squarings V <- V*V.
    All matmuls fp32 on the PE. PE computes lhsT.T @ rhs; Hermitian operands
    need no transposes (conj = negated imag plane); squarings use an explicit
    PE-transpose pass.
  - Host applies the 24 U_k to psi (0.005% of FLOPs) and returns |psi|^2.
"""

import math
from contextlib import ExitStack

import numpy as np

D = 1024           # statevector dim
P = 128            # partitions
NB = D // P        # 8 row blocks
CB = 512           # matmul moving free dim = one fp32 PSUM bank
NCOL = D // CB     # 2 col blocks
NK = 24            # gates
NCORES = 8
GPC = NK // NCORES # gates per core
DDEG = 11          # Taylor degree
LAM_BOUND = 64.3 * 1.06   # GUE edge 2*sqrt(D) with margin
X0 = 1.1           # max scaled norm after 2^-s scaling

_COEF = [(-1j) ** m / math.factorial(m) for m in range(DDEG + 1)]

_prog_cache = {}

# test-harness hooks: when TRACE is set, the SPMD run captures an NTFF
# profile and the BassKernelResults lands in LAST_RESULT.
TRACE = False
LAST_RESULT = None


def _build_program(slot_s):
    import concourse.bacc as bacc
    import concourse.tile as tile
    import concourse.mybir as mybir

    dt = mybir.dt
    f32 = dt.float32
    AL = mybir.AluOpType
    nslots = len(slot_s)

    nc = bacc.Bacc("TRN2", target_bir_lowering=False, debug=False,
                   num_devices=NCORES)

    rs_in = [nc.dram_tensor(f"rs{j}", [D, D], f32, kind="ExternalInput").ap()
             for j in range(nslots)]
    im_in = [nc.dram_tensor(f"gs{j}", [D, D], f32, kind="ExternalInput").ap()
             for j in range(nslots)]
    ident_in = nc.dram_tensor("ident", [P, P], f32, kind="ExternalInput").ap()
    u_out = [(nc.dram_tensor(f"u{j}re", [D, D], f32, kind="ExternalOutput").ap(),
              nc.dram_tensor(f"u{j}im", [D, D], f32, kind="ExternalOutput").ap())
             for j in range(nslots)]

    uid = [0]

    def nm(base):
        uid[0] += 1
        return f"{base}_{uid[0]}"

    def stage_src(plane):
        return plane.rearrange("(b p) c -> p b c", p=P)

    def col_src(plane, p0):
        return plane.rearrange("(kb q) m -> q kb m", q=P)[:, :, p0 * P:(p0 + 1) * P]

    with tile.TileContext(nc) as tc, ExitStack() as ctx:
        dram = ctx.enter_context(tc.tile_pool(name="dram", bufs=1, space="DRAM"))
        xst = ctx.enter_context(tc.tile_pool(name="xst", bufs=2))
        lst = ctx.enter_context(tc.tile_pool(name="lst", bufs=2))
        est = ctx.enter_context(tc.tile_pool(name="est", bufs=8))
        bst = ctx.enter_context(tc.tile_pool(name="bst", bufs=6))
        ps = ctx.enter_context(tc.tile_pool(name="ps", bufs=1, space="PSUM"))
        cst = ctx.enter_context(tc.tile_pool(name="cst", bufs=1))

        ident = cst.tile([P, P], f32, tag="ident", name="identt")
        nc.sync.dma_start(ident[:], ident_in)

        def dplane(tag):
            return dram.tile([D, D], f32, tag=tag, name=nm(tag))[:, :]

        def stage_plane(plane, tag):
            t = xst.tile([P, NB * D], f32, tag=tag, name=nm(tag))
            nc.sync.dma_start(t[:].rearrange("p (b c) -> p b c", b=NB),
                              stage_src(plane))
            return t

        def stage_cols(plane, p0, tag):
            t = lst.tile([P, NB * P], f32, tag=tag, name=nm(tag))
            nc.sync.dma_start(t[:].rearrange("p (b m) -> p b m", b=NB),
                              col_src(plane, p0))
            return t

        def psum_pair(idx):
            b0 = (2 * idx) % 8
            pr = ps.tile([P, CB], f32, tag=f"pb{b0}", name=nm("pr"))
            pi = ps.tile([P, CB], f32, tag=f"pb{b0 + 1}", name=nm("pi"))
            return pr, pi

        def matmul_c(L3, X2, evict):
            """C = (Lr + i*Li)^T @ (Xr + i*Xi). L3 = (Lr, Li, Li_neg) DRAM
            planes; X2 = (Xr, Xi). evict(p0, n, pr, pi) consumes PSUM tiles
            with C_re, C_im of output tile [p0*128:+128, n*512:+512]."""
            Lr_d, Li_d, Lin_d = L3
            Xr_d, Xi_d = X2
            xr = stage_plane(Xr_d, "xr")
            xi = stage_plane(Xi_d, "xi")
            for p0 in range(NB):
                lr = stage_cols(Lr_d, p0, "lr")
                li = stage_cols(Li_d, p0, "li")
                lin = stage_cols(Lin_d, p0, "lin")
                for n in range(NCOL):
                    pr, pi = psum_pair(p0 * NCOL + n)
                    c0 = n * CB
                    # C_re = Lr^T Xr - Li^T Xi  (PE(Lr,Xr) + PE(Li_neg,Xi))
                    for kb in range(NB):
                        nc.tensor.matmul(
                            pr[:], lr[:, kb * P:(kb + 1) * P],
                            xr[:, kb * D + c0: kb * D + c0 + CB],
                            start=(kb == 0), stop=False)
                    for kb in range(NB):
                        nc.tensor.matmul(
                            pr[:], lin[:, kb * P:(kb + 1) * P],
                            xi[:, kb * D + c0: kb * D + c0 + CB],
                            start=False, stop=(kb == NB - 1))
                    # C_im = Lr^T Xi + Li^T Xr
                    for kb in range(NB):
                        nc.tensor.matmul(
                            pi[:], lr[:, kb * P:(kb + 1) * P],
                            xi[:, kb * D + c0: kb * D + c0 + CB],
                            start=(kb == 0), stop=False)
                    for kb in range(NB):
                        nc.tensor.matmul(
                            pi[:], li[:, kb * P:(kb + 1) * P],
                            xr[:, kb * D + c0: kb * D + c0 + CB],
                            start=False, stop=(kb == NB - 1))
                    evict(p0, n, pr, pi)

        def osl(plane, p0, n):
            return plane[p0 * P:(p0 + 1) * P, n * CB:(n + 1) * CB]

        def plain_evict(out_planes, neg_plane=None):
            def ev(p0, n, pr, pi):
                sr = est.tile([P, CB], f32, tag="ev", name=nm("sr"))
                nc.vector.tensor_copy(sr[:], pr[:])
                nc.sync.dma_start(osl(out_planes[0], p0, n), sr[:])
                si = est.tile([P, CB], f32, tag="ev", name=nm("si"))
                nc.vector.tensor_copy(si[:], pi[:])
                nc.sync.dma_start(osl(out_planes[1], p0, n), si[:])
                if neg_plane is not None:
                    sn = est.tile([P, CB], f32, tag="ev", name=nm("sn"))
                    nc.vector.tensor_scalar_mul(sn[:], pi[:], -1.0)
                    nc.sync.dma_start(osl(neg_plane, p0, n), sn[:])
            return ev

        def chunk_evict(out_planes, cI, cM, cM2, Mpl, M2pl, diag_tiles):
            """Evict psum + (cI*I + cM*M + cM2*M2): the Horner '+ B_j'."""
            def ev(p0, n, pr, pi):
                outs = []
                for plane_i, pp in ((0, pr), (1, pi)):
                    if plane_i == 0:
                        terms = [(Mpl[0], cM.real), (Mpl[1], -cM.imag),
                                 (M2pl[0], cM2.real), (M2pl[1], -cM2.imag)]
                        dcoef = cI.real
                    else:
                        terms = [(Mpl[0], cM.imag), (Mpl[1], cM.real),
                                 (M2pl[0], cM2.imag), (M2pl[1], cM2.real)]
                        dcoef = cI.imag
                    terms = [(pl, cf) for (pl, cf) in terms if cf != 0.0]
                    cur = pp
                    for (pl, cf) in terms:
                        tt = bst.tile([P, CB], f32, tag="bt", name=nm("bt"))
                        nc.sync.dma_start(tt[:], osl(pl, p0, n))
                        st = est.tile([P, CB], f32, tag="ev", name=nm("hv"))
                        nc.vector.scalar_tensor_tensor(
                            st[:], tt[:], float(cf), cur[:],
                            op0=AL.mult, op1=AL.add)
                        cur = st
                    if cur is pp:
                        st = est.tile([P, CB], f32, tag="ev", name=nm("hc"))
                        nc.vector.tensor_copy(st[:], pp[:])
                        cur = st
                    if dcoef != 0.0 and n == p0 // (CB // P):
                        off = (p0 % (CB // P)) * P
                        nc.vector.tensor_add(cur[:, off:off + P],
                                             cur[:, off:off + P],
                                             diag_tiles[plane_i][:])
                    outs.append(cur)
                nc.sync.dma_start(osl(out_planes[0], p0, n), outs[0][:])
                nc.sync.dma_start(osl(out_planes[1], p0, n), outs[1][:])
            return ev

        def sym_pass(src_dram, dst, alu_op, neg_dst=None):
            """dst = src (op) src^T, blockwise via PE transpose."""
            S = stage_plane(src_dram, "xr")
            for bo in range(NB):
                for bi in range(NB):
                    pt = ps.tile([P, P], f32, tag=f"pb{(bo * NB + bi) % 8}",
                                 name=nm("pt"))
                    nc.tensor.transpose(
                        pt[:], S[:, bi * D + bo * P: bi * D + bo * P + P],
                        ident[:])
                    st = est.tile([P, P], f32, tag="tev", name=nm("st"))
                    nc.vector.tensor_tensor(
                        st[:], S[:, bo * D + bi * P: bo * D + bi * P + P],
                        pt[:], op=alu_op)
                    nc.sync.dma_start(
                        dst[bo * P:(bo + 1) * P, bi * P:(bi + 1) * P], st[:])
                    if neg_dst is not None:
                        sn = est.tile([P, P], f32, tag="tev", name=nm("sg"))
                        nc.vector.tensor_scalar_mul(sn[:], st[:], -1.0)
                        nc.sync.dma_start(
                            neg_dst[bo * P:(bo + 1) * P, bi * P:(bi + 1) * P],
                            sn[:])

        def transpose_pass(V2, T3):
            """(Tr, Ti, Ti_neg) = (Vr^T, Vi^T, -Vi^T)."""
            plans = [(V2[0], ((T3[0], False),)),
                     (V2[1], ((T3[1], False), (T3[2], True)))]
            for src, dsts in plans:
                S = stage_plane(src, "xr")
                for bo in range(NB):
                    for bi in range(NB):
                        pt = ps.tile([P, P], f32,
                                     tag=f"pb{(bo * NB + bi) % 8}",
                                     name=nm("pt"))
                        nc.tensor.transpose(
                            pt[:], S[:, bi * D + bo * P: bi * D + bo * P + P],
                            ident[:])
                        for (dstp, neg) in dsts:
                            st = est.tile([P, P], f32, tag="tev",
                                          name=nm("tt"))
                            if neg:
                                nc.vector.tensor_scalar_mul(st[:], pt[:],
                                                            -1.0)
                            else:
                                nc.vector.tensor_copy(st[:], pt[:])
                            nc.sync.dma_start(
                                dstp[bo * P:(bo + 1) * P,
                                     bi * P:(bi + 1) * P], st[:])

        def b3_pass(B3, Mpl, M2pl, diag_tiles):
            """B3 = c9*I + c10*M + c11*M2 built on DVE."""
            c9, c10, c11 = _COEF[9], _COEF[10], _COEF[11]
            for p0 in range(NB):
                for n in range(NCOL):
                    for plane_i in range(2):
                        if plane_i == 0:
                            t1 = ((Mpl[0], c10.real) if c10.real
                                  else (Mpl[1], -c10.imag))
                            t2 = ((M2pl[0], c11.real) if c11.real
                                  else (M2pl[1], -c11.imag))
                            dcoef = c9.real
                        else:
                            t1 = ((Mpl[1], c10.real) if c10.real
                                  else (Mpl[0], c10.imag))
                            t2 = ((M2pl[1], c11.real) if c11.real
                                  else (M2pl[0], c11.imag))
                            dcoef = c9.imag
                        a1 = bst.tile([P, CB], f32, tag="bt", name=nm("b1"))
                        a2 = bst.tile([P, CB], f32, tag="bt", name=nm("b2"))
                        nc.sync.dma_start(a1[:], osl(t1[0], p0, n))
                        nc.sync.dma_start(a2[:], osl(t2[0], p0, n))
                        s1 = est.tile([P, CB], f32, tag="ev", name=nm("b3a"))
                        nc.vector.tensor_scalar_mul(s1[:], a2[:],
                                                    float(t2[1]))
                        s2 = est.tile([P, CB], f32, tag="ev", name=nm("b3b"))
                        nc.vector.scalar_tensor_tensor(
                            s2[:], a1[:], float(t1[1]), s1[:],
                            op0=AL.mult, op1=AL.add)
                        if dcoef != 0.0 and n == p0 // (CB // P):
                            off = (p0 % (CB // P)) * P
                            nc.vector.tensor_add(s2[:, off:off + P],
                                                 s2[:, off:off + P],
                                                 diag_tiles[plane_i][:])
                        nc.sync.dma_start(osl(B3[plane_i], p0, n), s2[:])

        def make_diag_tiles(cI, tagbase):
            tiles = []
            for plane_i, v in enumerate((cI.real, cI.imag)):
                t = cst.tile([P, P], f32, tag=f"{tagbase}{plane_i}",
                             name=nm("dg"))
                if v != 0.0:
                    nc.vector.tensor_scalar_mul(t[:], ident[:], float(v))
                else:
                    nc.vector.memset(t[:], 0.0)
                tiles.append(t)
            return tiles

        # ---------------- per-gate flow ----------------
        for j, s in enumerate(slot_s):
            Mr, Mi, Min = dplane("mr"), dplane("mi"), dplane("min")
            M2r, M2i = dplane("m2r"), dplane("m2i")
            M3r, M3i, M3in = dplane("m3r"), dplane("m3i"), dplane("m3in")
            B3r, B3i = dplane("b3r"), dplane("b3i")
            Qr = [dplane("qar"), dplane("qbr")]
            Qi = [dplane("qai"), dplane("qbi")]
            Tr, Ti, Tin = dplane("tr"), dplane("ti"), dplane("tin")

            sym_pass(rs_in[j], Mr, AL.add)
            sym_pass(im_in[j], Mi, AL.subtract, neg_dst=Min)

            # M2 = M @ M   (lhsT = conj(M) = (Mr, -Mi) -> pass (Mr, Min, Mi))
            matmul_c((Mr, Min, Mi), (Mr, Mi), plain_evict((M2r, M2i)))
            # M3 = M @ M2  (stored with negated imag for later lhsT use)
            matmul_c((Mr, Min, Mi), (M2r, M2i),
                     plain_evict((M3r, M3i), neg_plane=M3in))

            dg3 = make_diag_tiles(_COEF[9], "dg3_")
            b3_pass((B3r, B3i), (Mr, Mi), (M2r, M2i), dg3)

            # Horner: Q = B3; for jc in (2,1,0): Q = M3 @ Q + B_jc
            prev = (B3r, B3i)
            for t_i, jc in enumerate((2, 1, 0)):
                if t_i == 2 and s == 0:
                    tgt = u_out[j]
                else:
                    tgt = (Qr[t_i % 2], Qi[t_i % 2])
                dg = make_diag_tiles(_COEF[3 * jc], f"dgh{t_i}_")
                matmul_c((M3r, M3in, M3i), prev,
                         chunk_evict(tgt, _COEF[3 * jc], _COEF[3 * jc + 1],
                                     _COEF[3 * jc + 2], (Mr, Mi),
                                     (M2r, M2i), dg))
                prev = tgt

            # squarings: V <- V @ V, s times; last lands in u_out[j]
            V = prev
            other = (Qr[1], Qi[1])
            for t in range(s):
                transpose_pass(V, (Tr, Ti, Tin))
                out_pl = u_out[j] if t == s - 1 else other
                matmul_c((Tr, Ti, Tin), V, plain_evict(out_pl))
                V, other = out_pl, V

    nc.compile()
    return nc


def _get_program(slot_s):
    key = tuple(slot_s)
    if key not in _prog_cache:
        _prog_cache[key] = _build_program(key)
    return _prog_cache[key]


def _plan(th):
    a = np.abs(th) * LAM_BOUND
    order = np.argsort(-a)          # rank -> gate index
    slot_s = []
    for j in range(GPC):
        grp = a[order[j * NCORES:(j + 1) * NCORES]]
        s = max(0, math.ceil(math.log2(max(float(grp.max()), 1e-9) / X0)))
        slot_s.append(int(s))
    return order, slot_s


def kernel(feature, theta, gens_re, gens_im):
    feature = np.asarray(feature)
    th = np.asarray(theta)[:, 0].astype(np.float64)
    gens_re = np.asarray(gens_re)
    gens_im = np.asarray(gens_im)

    order, slot_s = _plan(th)
    nc = _get_program(tuple(slot_s))

    ident = np.eye(P, dtype=np.float32)
    in_maps = []
    for c in range(NCORES):
        m = {"ident": ident}
        for j in range(GPC):
            k = int(order[j * NCORES + c])
            cc = 0.5 * th[k] / (2.0 ** slot_s[j])
            m[f"rs{j}"] = np.ascontiguousarray(cc * gens_re[k],
                                               dtype=np.float32)
            m[f"gs{j}"] = np.ascontiguousarray(cc * gens_im[k],
                                               dtype=np.float32)
        in_maps.append(m)

    from concourse.bass_utils import run_bass_kernel_spmd
    res = run_bass_kernel_spmd(nc, in_maps, core_ids=list(range(NCORES)),
                               trace=TRACE)
    global LAST_RESULT
    LAST_RESULT = res

    U = {}
    for c in range(NCORES):
        for j in range(GPC):
            k = int(order[j * NCORES + c])
            U[k] = (res.results[c][f"u{j}re"].astype(np.float64)
                    + 1j * res.results[c][f"u{j}im"].astype(np.float64))

    psi = np.zeros(D, np.complex128)
    psi[:feature.shape[0]] = feature.astype(np.float64)
    psi /= np.linalg.norm(psi)
    for k in range(NK):
        psi = U[k] @ psi
    return (np.abs(psi) ** 2).astype(np.float32)


# revision 8
# speedup vs baseline: 1.1172x; 1.1172x over previous
"""Trainium2 Bass kernel for nn_DFTQNN_81776177316168.

reference: probs = |U_24 ... U_1 psi|^2 with U_k = expm(-i theta_k G_k),
G_k = (M_k + M_k^H)/2 Hermitian 1024x1024, psi = normalized padded feature.

Strategy (expert-parallel, per sharding hint):
  - 24 gates across 8 cores, 3 per core. Gates ranked by spectral-norm bound
    a_k = |theta_k| * lam_bound; slot j of every core holds ranks j*8..j*8+7,
    so slot squaring count s_j is uniform across cores (SPMD single program).
  - Per gate on device: M = (theta/2^s) G (built by PE-transpose symmetrize),
    V0 = exp(-iM) by degree-11 Taylor in Paterson-Stockmeyer form
    (powers M^2, M^3, then 3 Horner steps with the per-chunk linear term
    fused into the PSUM eviction), then s repeated squarings V <- V*V.
    All matmuls fp32 on the PE. PE computes lhsT.T @ rhs; Hermitian operands
    need no transposes (conj = negated imag plane); squarings use an explicit
    PE-transpose pass.
  - Host applies the 24 U_k to psi (0.005% of FLOPs) and returns |psi|^2.
"""

import math
from contextlib import ExitStack

import numpy as np

D = 1024           # statevector dim
P = 128            # partitions
NB = D // P        # 8 row blocks
CB = 512           # matmul moving free dim = one fp32 PSUM bank
NCOL = D // CB     # 2 col blocks
NK = 24            # gates
NCORES = 8
GPC = NK // NCORES # gates per core
DDEG = 11          # Taylor degree
LAM_BOUND = 64.3 * 1.06   # GUE edge 2*sqrt(D) with margin
X0 = 1.5           # max scaled norm after 2^-s scaling

_COEF = [(-1j) ** m / math.factorial(m) for m in range(DDEG + 1)]

_prog_cache = {}

# test-harness hooks: when TRACE is set, the SPMD run captures an NTFF
# profile and the BassKernelResults lands in LAST_RESULT.
TRACE = False
LAST_RESULT = None


def _build_program(slot_s):
    import concourse.bacc as bacc
    import concourse.tile as tile
    import concourse.mybir as mybir

    dt = mybir.dt
    f32 = dt.float32
    AL = mybir.AluOpType
    nslots = len(slot_s)

    nc = bacc.Bacc("TRN2", target_bir_lowering=False, debug=False,
                   num_devices=NCORES)

    rs_in = [nc.dram_tensor(f"rs{j}", [D, D], f32, kind="ExternalInput").ap()
             for j in range(nslots)]
    im_in = [nc.dram_tensor(f"gs{j}", [D, D], f32, kind="ExternalInput").ap()
             for j in range(nslots)]
    ident_in = nc.dram_tensor("ident", [P, P], f32, kind="ExternalInput").ap()
    u_out = [(nc.dram_tensor(f"u{j}re", [D, D], f32, kind="ExternalOutput").ap(),
              nc.dram_tensor(f"u{j}im", [D, D], f32, kind="ExternalOutput").ap())
             for j in range(nslots)]

    uid = [0]

    def nm(base):
        uid[0] += 1
        return f"{base}_{uid[0]}"

    def stage_src(plane):
        return plane.rearrange("(b p) c -> p b c", p=P)

    def col_src(plane, p0):
        return plane.rearrange("(kb q) m -> q kb m", q=P)[:, :, p0 * P:(p0 + 1) * P]

    with tile.TileContext(nc) as tc, ExitStack() as ctx:
        dram = ctx.enter_context(tc.tile_pool(name="dram", bufs=1, space="DRAM"))
        xst = ctx.enter_context(tc.tile_pool(name="xst", bufs=2))
        lst = ctx.enter_context(tc.tile_pool(name="lst", bufs=2))
        est = ctx.enter_context(tc.tile_pool(name="est", bufs=8))
        bst = ctx.enter_context(tc.tile_pool(name="bst", bufs=6))
        ps = ctx.enter_context(tc.tile_pool(name="ps", bufs=1, space="PSUM"))
        cst = ctx.enter_context(tc.tile_pool(name="cst", bufs=1))

        ident = cst.tile([P, P], f32, tag="ident", name="identt")
        nc.sync.dma_start(ident[:], ident_in)

        def dplane(tag):
            return dram.tile([D, D], f32, tag=tag, name=nm(tag))[:, :]

        def stage_plane(plane, tag):
            # chunked per row-block so restaging overlaps with the producer's
            # evictions (per-region DRAM deps) instead of waiting for the
            # whole plane
            t = xst.tile([P, NB * D], f32, tag=tag, name=nm(tag))
            for kb in range(NB):
                nc.sync.dma_start(t[:, kb * D:(kb + 1) * D],
                                  plane[kb * P:(kb + 1) * P, :])
            return t

        def stage_cols(plane, p0, tag):
            t = lst.tile([P, NB * P], f32, tag=tag, name=nm(tag))
            nc.sync.dma_start(t[:].rearrange("p (b m) -> p b m", b=NB),
                              col_src(plane, p0))
            return t

        def psum_pair(idx):
            b0 = (2 * idx) % 8
            pr = ps.tile([P, CB], f32, tag=f"pb{b0}", name=nm("pr"))
            pi = ps.tile([P, CB], f32, tag=f"pb{b0 + 1}", name=nm("pi"))
            return pr, pi

        def matmul_c(L3, X2, evict):
            """C = (Lr + i*Li)^T @ (Xr + i*Xi). L3 = (Lr, Li, Li_neg) DRAM
            planes; X2 = (Xr, Xi). evict(p0, n, pr, pi) consumes PSUM tiles
            with C_re, C_im of output tile [p0*128:+128, n*512:+512]."""
            Lr_d, Li_d, Lin_d = L3
            Xr_d, Xi_d = X2
            xr = stage_plane(Xr_d, "xr")
            xi = stage_plane(Xi_d, "xi")
            for p0 in range(NB):
                lr = stage_cols(Lr_d, p0, "lr")
                li = stage_cols(Li_d, p0, "li")
                lin = stage_cols(Lin_d, p0, "lin")
                for n in range(NCOL):
                    pr, pi = psum_pair(p0 * NCOL + n)
                    c0 = n * CB
                    # C_re = Lr^T Xr - Li^T Xi  (PE(Lr,Xr) + PE(Li_neg,Xi))
                    for kb in range(NB):
                        nc.tensor.matmul(
                            pr[:], lr[:, kb * P:(kb + 1) * P],
                            xr[:, kb * D + c0: kb * D + c0 + CB],
                            start=(kb == 0), stop=False)
                    for kb in range(NB):
                        nc.tensor.matmul(
                            pr[:], lin[:, kb * P:(kb + 1) * P],
                            xi[:, kb * D + c0: kb * D + c0 + CB],
                            start=False, stop=(kb == NB - 1))
                    # C_im = Lr^T Xi + Li^T Xr
                    for kb in range(NB):
                        nc.tensor.matmul(
                            pi[:], lr[:, kb * P:(kb + 1) * P],
                            xi[:, kb * D + c0: kb * D + c0 + CB],
                            start=(kb == 0), stop=False)
                    for kb in range(NB):
                        nc.tensor.matmul(
                            pi[:], li[:, kb * P:(kb + 1) * P],
                            xr[:, kb * D + c0: kb * D + c0 + CB],
                            start=False, stop=(kb == NB - 1))
                    evict(p0, n, pr, pi)

        def osl(plane, p0, n):
            return plane[p0 * P:(p0 + 1) * P, n * CB:(n + 1) * CB]

        def plain_evict(out_planes, neg_plane=None):
            def ev(p0, n, pr, pi):
                sr = est.tile([P, CB], f32, tag="ev", name=nm("sr"))
                nc.vector.tensor_copy(sr[:], pr[:])
                nc.sync.dma_start(osl(out_planes[0], p0, n), sr[:])
                si = est.tile([P, CB], f32, tag="ev", name=nm("si"))
                nc.vector.tensor_copy(si[:], pi[:])
                nc.sync.dma_start(osl(out_planes[1], p0, n), si[:])
                if neg_plane is not None:
                    sn = est.tile([P, CB], f32, tag="ev", name=nm("sn"))
                    nc.vector.tensor_scalar_mul(sn[:], pi[:], -1.0)
                    nc.sync.dma_start(osl(neg_plane, p0, n), sn[:])
            return ev

        def chunk_evict(out_planes, cI, cM, cM2, Mpl, M2pl, diag_tiles):
            """Evict psum + (cI*I + cM*M + cM2*M2): the Horner '+ B_j'."""
            def ev(p0, n, pr, pi):
                outs = []
                for plane_i, pp in ((0, pr), (1, pi)):
                    if plane_i == 0:
                        terms = [(Mpl[0], cM.real), (Mpl[1], -cM.imag),
                                 (M2pl[0], cM2.real), (M2pl[1], -cM2.imag)]
                        dcoef = cI.real
                    else:
                        terms = [(Mpl[0], cM.imag), (Mpl[1], cM.real),
                                 (M2pl[0], cM2.imag), (M2pl[1], cM2.real)]
                        dcoef = cI.imag
                    terms = [(pl, cf) for (pl, cf) in terms if cf != 0.0]
                    cur = pp
                    for (pl, cf) in terms:
                        tt = bst.tile([P, CB], f32, tag="bt", name=nm("bt"))
                        nc.sync.dma_start(tt[:], osl(pl, p0, n))
                        st = est.tile([P, CB], f32, tag="ev", name=nm("hv"))
                        nc.vector.scalar_tensor_tensor(
                            st[:], tt[:], float(cf), cur[:],
                            op0=AL.mult, op1=AL.add)
                        cur = st
                    if cur is pp:
                        st = est.tile([P, CB], f32, tag="ev", name=nm("hc"))
                        nc.vector.tensor_copy(st[:], pp[:])
                        cur = st
                    if dcoef != 0.0 and n == p0 // (CB // P):
                        off = (p0 % (CB // P)) * P
                        nc.vector.tensor_add(cur[:, off:off + P],
                                             cur[:, off:off + P],
                                             diag_tiles[plane_i][:])
                    outs.append(cur)
                nc.sync.dma_start(osl(out_planes[0], p0, n), outs[0][:])
                nc.sync.dma_start(osl(out_planes[1], p0, n), outs[1][:])
            return ev

        def sym_pass(src_dram, dst, alu_op, neg_dst=None):
            """dst = src (op) src^T, blockwise via PE transpose."""
            S = stage_plane(src_dram, "xr")
            for bo in range(NB):
                for bi in range(NB):
                    pt = ps.tile([P, P], f32, tag=f"pb{(bo * NB + bi) % 8}",
                                 name=nm("pt"))
                    nc.tensor.transpose(
                        pt[:], S[:, bi * D + bo * P: bi * D + bo * P + P],
                        ident[:])
                    st = est.tile([P, P], f32, tag="tev", name=nm("st"))
                    nc.vector.tensor_tensor(
                        st[:], S[:, bo * D + bi * P: bo * D + bi * P + P],
                        pt[:], op=alu_op)
                    nc.sync.dma_start(
                        dst[bo * P:(bo + 1) * P, bi * P:(bi + 1) * P], st[:])
                    if neg_dst is not None:
                        sn = est.tile([P, P], f32, tag="tev", name=nm("sg"))
                        nc.vector.tensor_scalar_mul(sn[:], st[:], -1.0)
                        nc.sync.dma_start(
                            neg_dst[bo * P:(bo + 1) * P, bi * P:(bi + 1) * P],
                            sn[:])

        def transpose_pass(V2, T3):
            """(Tr, Ti, Ti_neg) = (Vr^T, Vi^T, -Vi^T)."""
            plans = [(V2[0], ((T3[0], False),)),
                     (V2[1], ((T3[1], False), (T3[2], True)))]
            for src, dsts in plans:
                S = stage_plane(src, "xr")
                for bo in range(NB):
                    for bi in range(NB):
                        pt = ps.tile([P, P], f32,
                                     tag=f"pb{(bo * NB + bi) % 8}",
                                     name=nm("pt"))
                        nc.tensor.transpose(
                            pt[:], S[:, bi * D + bo * P: bi * D + bo * P + P],
                            ident[:])
                        for (dstp, neg) in dsts:
                            st = est.tile([P, P], f32, tag="tev",
                                          name=nm("tt"))
                            if neg:
                                nc.vector.tensor_scalar_mul(st[:], pt[:],
                                                            -1.0)
                            else:
                                nc.vector.tensor_copy(st[:], pt[:])
                            nc.sync.dma_start(
                                dstp[bo * P:(bo + 1) * P,
                                     bi * P:(bi + 1) * P], st[:])

        def emit_b3_tile(p0, n, B3, Mpl, M2pl, diag_tiles):
            """B3[p0,n] = (c9*I + c10*M + c11*M2)[p0,n] built on DVE."""
            c9, c10, c11 = _COEF[9], _COEF[10], _COEF[11]
            if True:
                if True:
                    for plane_i in range(2):
                        if plane_i == 0:
                            t1 = ((Mpl[0], c10.real) if c10.real
                                  else (Mpl[1], -c10.imag))
                            t2 = ((M2pl[0], c11.real) if c11.real
                                  else (M2pl[1], -c11.imag))
                            dcoef = c9.real
                        else:
                            t1 = ((Mpl[1], c10.real) if c10.real
                                  else (Mpl[0], c10.imag))
                            t2 = ((M2pl[1], c11.real) if c11.real
                                  else (M2pl[0], c11.imag))
                            dcoef = c9.imag
                        a1 = bst.tile([P, CB], f32, tag="bt", name=nm("b1"))
                        a2 = bst.tile([P, CB], f32, tag="bt", name=nm("b2"))
                        nc.sync.dma_start(a1[:], osl(t1[0], p0, n))
                        nc.sync.dma_start(a2[:], osl(t2[0], p0, n))
                        s1 = est.tile([P, CB], f32, tag="ev", name=nm("b3a"))
                        nc.vector.tensor_scalar_mul(s1[:], a2[:],
                                                    float(t2[1]))
                        s2 = est.tile([P, CB], f32, tag="ev", name=nm("b3b"))
                        nc.vector.scalar_tensor_tensor(
                            s2[:], a1[:], float(t1[1]), s1[:],
                            op0=AL.mult, op1=AL.add)
                        if dcoef != 0.0 and n == p0 // (CB // P):
                            off = (p0 % (CB // P)) * P
                            nc.vector.tensor_add(s2[:, off:off + P],
                                                 s2[:, off:off + P],
                                                 diag_tiles[plane_i][:])
                        nc.sync.dma_start(osl(B3[plane_i], p0, n), s2[:])

        def make_diag_tiles(cI, tagbase):
            tiles = []
            for plane_i, v in enumerate((cI.real, cI.imag)):
                t = cst.tile([P, P], f32, tag=f"{tagbase}{plane_i}",
                             name=nm("dg"))
                if v != 0.0:
                    nc.vector.tensor_scalar_mul(t[:], ident[:], float(v))
                else:
                    nc.vector.memset(t[:], 0.0)
                tiles.append(t)
            return tiles

        # ---------------- per-gate flow ----------------
        for j, s in enumerate(slot_s):
            Mr, Mi, Min = dplane("mr"), dplane("mi"), dplane("min")
            M2r, M2i = dplane("m2r"), dplane("m2i")
            M3r, M3i, M3in = dplane("m3r"), dplane("m3i"), dplane("m3in")
            B3r, B3i = dplane("b3r"), dplane("b3i")
            Qr = [dplane("qar"), dplane("qbr")]
            Qi = [dplane("qai"), dplane("qbi")]
            Tr, Ti, Tin = dplane("tr"), dplane("ti"), dplane("tin")

            sym_pass(rs_in[j], Mr, AL.add)
            sym_pass(im_in[j], Mi, AL.subtract, neg_dst=Min)

            # M2 = M @ M   (lhsT = conj(M) = (Mr, -Mi) -> pass (Mr, Min, Mi))
            matmul_c((Mr, Min, Mi), (Mr, Mi), plain_evict((M2r, M2i)))
            # M3 = M @ M2  (stored with negated imag for later lhsT use);
            # B3 tiles ride along in the same eviction cadence so the PE
            # never idles on a DVE-only pass
            dg3 = make_diag_tiles(_COEF[9], "dg3_")
            _m3ev = plain_evict((M3r, M3i), neg_plane=M3in)

            def m3b3_ev(p0, n, pr, pi):
                _m3ev(p0, n, pr, pi)
                emit_b3_tile(p0, n, (B3r, B3i), (Mr, Mi), (M2r, M2i), dg3)

            matmul_c((Mr, Min, Mi), (M2r, M2i), m3b3_ev)

            # Horner: Q = B3; for jc in (2,1,0): Q = M3 @ Q + B_jc
            prev = (B3r, B3i)
            for t_i, jc in enumerate((2, 1, 0)):
                if t_i == 2 and s == 0:
                    tgt = u_out[j]
                else:
                    tgt = (Qr[t_i % 2], Qi[t_i % 2])
                dg = make_diag_tiles(_COEF[3 * jc], f"dgh{t_i}_")
                matmul_c((M3r, M3in, M3i), prev,
                         chunk_evict(tgt, _COEF[3 * jc], _COEF[3 * jc + 1],
                                     _COEF[3 * jc + 2], (Mr, Mi),
                                     (M2r, M2i), dg))
                prev = tgt

            # squarings: V <- V @ V, s times; last lands in u_out[j]
            V = prev
            other = (Qr[1], Qi[1])
            for t in range(s):
                transpose_pass(V, (Tr, Ti, Tin))
                out_pl = u_out[j] if t == s - 1 else other
                matmul_c((Tr, Ti, Tin), V, plain_evict(out_pl))
                V, other = out_pl, V

    nc.compile()
    return nc


def _get_program(slot_s):
    key = tuple(slot_s)
    if key not in _prog_cache:
        _prog_cache[key] = _build_program(key)
    return _prog_cache[key]


def _plan(th):
    a = np.abs(th) * LAM_BOUND
    order = np.argsort(-a)          # rank -> gate index
    slot_s = []
    for j in range(GPC):
        grp = a[order[j * NCORES:(j + 1) * NCORES]]
        s = max(0, math.ceil(math.log2(max(float(grp.max()), 1e-9) / X0)))
        slot_s.append(int(s))
    return order, slot_s


def kernel(feature, theta, gens_re, gens_im):
    feature = np.asarray(feature)
    th = np.asarray(theta)[:, 0].astype(np.float64)
    gens_re = np.asarray(gens_re)
    gens_im = np.asarray(gens_im)

    order, slot_s = _plan(th)
    nc = _get_program(tuple(slot_s))

    ident = np.eye(P, dtype=np.float32)
    in_maps = []
    for c in range(NCORES):
        m = {"ident": ident}
        for j in range(GPC):
            k = int(order[j * NCORES + c])
            cc = 0.5 * th[k] / (2.0 ** slot_s[j])
            m[f"rs{j}"] = np.ascontiguousarray(cc * gens_re[k],
                                               dtype=np.float32)
            m[f"gs{j}"] = np.ascontiguousarray(cc * gens_im[k],
                                               dtype=np.float32)
        in_maps.append(m)

    from concourse.bass_utils import run_bass_kernel_spmd
    res = run_bass_kernel_spmd(nc, in_maps, core_ids=list(range(NCORES)),
                               trace=TRACE)
    global LAST_RESULT
    LAST_RESULT = res

    U = {}
    for c in range(NCORES):
        for j in range(GPC):
            k = int(order[j * NCORES + c])
            U[k] = (res.results[c][f"u{j}re"].astype(np.float64)
                    + 1j * res.results[c][f"u{j}im"].astype(np.float64))

    psi = np.zeros(D, np.complex128)
    psi[:feature.shape[0]] = feature.astype(np.float64)
    psi /= np.linalg.norm(psi)
    for k in range(NK):
        psi = U[k] @ psi
    return (np.abs(psi) ** 2).astype(np.float32)


# revision 10
# speedup vs baseline: 1.2606x; 1.1283x over previous
"""Trainium2 Bass kernel for nn_DFTQNN_81776177316168.

reference: probs = |U_24 ... U_1 psi|^2 with U_k = expm(-i theta_k G_k),
G_k Hermitian 1024x1024 (symmetrized complex gaussian), psi = normalized
padded feature.

Strategy (expert-parallel, per the sharding hint):
  - 24 gates across 8 cores, 3 per core. Gates ranked by the spectral-norm
    bound a_k = |theta_k| * lam_bound; slot j of every core holds ranks
    j*8..j*8+7, so the slot squaring count s_j is uniform across cores
    (single SPMD program).
  - Per gate on device: M = (theta/2^s) G, V0 = exp(-iM) by degree-11
    Taylor in Paterson-Stockmeyer form (powers M^2, M^3, then 3 Horner
    steps whose '+B_j' linear term is fused into the PSUM eviction), then
    s repeated squarings V <- V*V.
  - Matmuls run as fp16 hi/lo split pairs (Dekker): X = X_h + X_l/2048
    with X_h, X_l fp16. A product A*B = A_h B_h + (A_h B_l + A_l B_h)/2048
    accumulates the main and cross terms in separate PSUM banks (fp32) and
    combines on the DVE at eviction; ~2^-22 relative error at 3x the fp32
    PE throughput. PE computes lhsT.T @ rhs; Hermitian operands need no
    transposes (conj = negated imag plane); squarings use a PE-transpose
    pass (transposing the fp16 planes is lossless).
  - Host symmetrizes/scales the generators, splits to fp16 pairs, and at
    the end applies the 24 U_k to psi (0.005% of FLOPs) -> |psi|^2.
"""

import math
from contextlib import ExitStack

import numpy as np

D = 1024           # statevector dim
P = 128            # partitions
NB = D // P        # 8 row blocks
CB = 512           # matmul moving free dim = one fp32 PSUM bank
NCOL = D // CB     # 2 col blocks
NK = 24            # gates
NCORES = 8
GPC = NK // NCORES # gates per core
DDEG = 11          # Taylor degree
LAM_BOUND = 64.3 * 1.06   # GUE edge 2*sqrt(D) with margin
X0 = 1.5           # max scaled norm after 2^-s scaling
LOSC = 2048.0      # lo-plane scale (2^11)

_COEF = [(-1j) ** m / math.factorial(m) for m in range(DDEG + 1)]

_prog_cache = {}

# test-harness hooks: when TRACE is set, the SPMD run captures an NTFF
# profile and the BassKernelResults lands in LAST_RESULT.
TRACE = False
LAST_RESULT = None

IN_NAMES = ("mrh", "mrl", "mih", "mil", "mnh", "mnl")


def _build_program(slot_s):
    import concourse.bacc as bacc
    import concourse.tile as tile
    import concourse.mybir as mybir

    dt = mybir.dt
    f32 = dt.float32
    f16 = dt.float16
    AL = mybir.AluOpType
    nslots = len(slot_s)

    nc = bacc.Bacc("TRN2", target_bir_lowering=False, debug=False,
                   num_devices=NCORES)

    m_in = [{nmm: nc.dram_tensor(f"{nmm}{j}", [D, D], f16,
                                 kind="ExternalInput").ap()
             for nmm in IN_NAMES} for j in range(nslots)]
    ident_in = nc.dram_tensor("ident", [P, P], f32, kind="ExternalInput").ap()
    u_out = [(nc.dram_tensor(f"u{j}re", [D, D], f32, kind="ExternalOutput").ap(),
              nc.dram_tensor(f"u{j}im", [D, D], f32, kind="ExternalOutput").ap())
             for j in range(nslots)]

    uid = [0]

    def nm(base):
        uid[0] += 1
        return f"{base}_{uid[0]}"

    def col_src(plane, p0):
        return plane.rearrange("(kb q) m -> q kb m", q=P)[:, :,
                                                          p0 * P:(p0 + 1) * P]

    with tile.TileContext(nc) as tc, ExitStack() as ctx:
        dram = ctx.enter_context(tc.tile_pool(name="dram", bufs=1,
                                              space="DRAM"))
        xst = ctx.enter_context(tc.tile_pool(name="xst", bufs=2))
        lst = ctx.enter_context(tc.tile_pool(name="lst", bufs=2))
        est = ctx.enter_context(tc.tile_pool(name="est", bufs=8))
        bst = ctx.enter_context(tc.tile_pool(name="bst", bufs=6))
        ps = ctx.enter_context(tc.tile_pool(name="ps", bufs=1, space="PSUM"))
        cst = ctx.enter_context(tc.tile_pool(name="cst", bufs=1))

        ident = cst.tile([P, P], f32, tag="ident", name="identt")
        nc.sync.dma_start(ident[:], ident_in)
        ident16 = cst.tile([P, P], f16, tag="ident16", name="identt16")
        nc.vector.tensor_copy(ident16[:], ident[:])

        def dplane(tag):
            return dram.tile([D, D], f16, tag=tag, name=nm(tag))[:, :]

        def stage_plane(plane, tag):
            # chunked per row-block so restaging overlaps the producer's
            # evictions (per-region DRAM deps)
            t = xst.tile([P, NB * D], f16, tag=tag, name=nm(tag))
            for kb in range(NB):
                nc.sync.dma_start(t[:, kb * D:(kb + 1) * D],
                                  plane[kb * P:(kb + 1) * P, :])
            return t

        def stage_cols(plane, p0, tag):
            t = lst.tile([P, NB * P], f16, tag=tag, name=nm(tag))
            nc.sync.dma_start(t[:].rearrange("p (b m) -> p b m", b=NB),
                              col_src(plane, p0))
            return t

        def psum_quad(idx):
            b0 = (4 * idx) % 8
            return [ps.tile([P, CB], f32, tag=f"pb{b0 + i}", name=nm("pq"))
                    for i in range(4)]

        def matmul_c(L6, X4, evict):
            """C = L^T @ X complex, fp16-split. L6 = (Lr_h, Lr_l, Li_h,
            Li_l, Lin_h, Lin_l) DRAM planes (Lin = -Li); X4 = (Xr_h, Xr_l,
            Xi_h, Xi_l). evict(p0, n, Ar, Br, Ai, Bi) consumes PSUM:
            C_plane = A + B/2048."""
            xrh = stage_plane(X4[0], "xrh")
            xrl = stage_plane(X4[1], "xrl")
            xih = stage_plane(X4[2], "xih")
            xil = stage_plane(X4[3], "xil")
            for p0 in range(NB):
                lrh = stage_cols(L6[0], p0, "lrh")
                lrl = stage_cols(L6[1], p0, "lrl")
                lih = stage_cols(L6[2], p0, "lih")
                lil = stage_cols(L6[3], p0, "lil")
                lnh = stage_cols(L6[4], p0, "lnh")
                lnl = stage_cols(L6[5], p0, "lnl")
                for n in range(NCOL):
                    Ar, Br, Ai, Bi = psum_quad(p0 * NCOL + n)
                    c0 = n * CB

                    def seqs(bank, pairs):
                        last = len(pairs) * NB - 1
                        i = 0
                        for (lt, xt) in pairs:
                            for kb in range(NB):
                                nc.tensor.matmul(
                                    bank[:], lt[:, kb * P:(kb + 1) * P],
                                    xt[:, kb * D + c0: kb * D + c0 + CB],
                                    start=(i == 0), stop=(i == last))
                                i += 1

                    # C_re = Lr^T Xr - Li^T Xi ; minus folded via Lin
                    seqs(Ar, [(lrh, xrh), (lnh, xih)])
                    seqs(Br, [(lrh, xrl), (lrl, xrh), (lnh, xil),
                              (lnl, xih)])
                    # C_im = Lr^T Xi + Li^T Xr
                    seqs(Ai, [(lrh, xih), (lih, xrh)])
                    seqs(Bi, [(lrh, xil), (lrl, xih), (lih, xrl),
                              (lil, xrh)])
                    evict(p0, n, Ar, Br, Ai, Bi)

        def osl(plane, p0, n):
            return plane[p0 * P:(p0 + 1) * P, n * CB:(n + 1) * CB]

        def combine(A, B):
            """fp32 staging tile = A + B/2048 from the two PSUM banks."""
            t0 = est.tile([P, CB], f32, tag="ev", name=nm("cb"))
            nc.vector.tensor_copy(t0[:], A[:])
            t1 = est.tile([P, CB], f32, tag="ev", name=nm("cc"))
            nc.vector.scalar_tensor_tensor(t1[:], B[:], 1.0 / LOSC, t0[:],
                                           op0=AL.mult, op1=AL.add)
            return t1

        def split_out(t, planes, p0, n, neg_planes=None):
            """Write fp32 tile t as fp16 (hi, lo*2048) planes (+negated)."""
            hi = est.tile([P, CB], f16, tag="evh", name=nm("hi"))
            nc.vector.tensor_copy(hi[:], t[:])
            nc.sync.dma_start(osl(planes[0], p0, n), hi[:])
            r = est.tile([P, CB], f32, tag="ev", name=nm("rr"))
            nc.vector.scalar_tensor_tensor(r[:], hi[:], -1.0, t[:],
                                           op0=AL.mult, op1=AL.add)
            lo = est.tile([P, CB], f16, tag="evh", name=nm("lo"))
            nc.vector.tensor_scalar_mul(lo[:], r[:], LOSC)
            nc.sync.dma_start(osl(planes[1], p0, n), lo[:])
            if neg_planes is not None:
                nh = est.tile([P, CB], f16, tag="evh", name=nm("nh"))
                nc.vector.tensor_scalar_mul(nh[:], hi[:], -1.0)
                nc.sync.dma_start(osl(neg_planes[0], p0, n), nh[:])
            if neg_planes is not None:
                nl = est.tile([P, CB], f16, tag="evh", name=nm("nl"))
                nc.vector.tensor_scalar_mul(nl[:], lo[:], -1.0)
                nc.sync.dma_start(osl(neg_planes[1], p0, n), nl[:])

        def add_terms(t, p0, n, terms):
            """t += sum coef*plane over fp16 (hi, lo) DRAM plane pairs."""
            cur = t
            for (ph, pl, cf) in terms:
                th = bst.tile([P, CB], f16, tag="bt", name=nm("th"))
                nc.sync.dma_start(th[:], osl(ph, p0, n))
                s1 = est.tile([P, CB], f32, tag="ev", name=nm("s1"))
                nc.vector.scalar_tensor_tensor(s1[:], th[:], float(cf),
                                               cur[:], op0=AL.mult,
                                               op1=AL.add)
                tl = bst.tile([P, CB], f16, tag="bt", name=nm("tl"))
                nc.sync.dma_start(tl[:], osl(pl, p0, n))
                s2 = est.tile([P, CB], f32, tag="ev", name=nm("s2"))
                nc.vector.scalar_tensor_tensor(s2[:], tl[:],
                                               float(cf) / LOSC, s1[:],
                                               op0=AL.mult, op1=AL.add)
                cur = s2
            return cur

        def add_diag(t, p0, n, dtile):
            if n == p0 // (CB // P):
                off = (p0 % (CB // P)) * P
                nc.vector.tensor_add(t[:, off:off + P], t[:, off:off + P],
                                     dtile[:])

        def make_diag_tiles(cI, tagbase):
            tiles = []
            for plane_i, v in enumerate((cI.real, cI.imag)):
                t = cst.tile([P, P], f32, tag=f"{tagbase}{plane_i}",
                             name=nm("dg"))
                if v != 0.0:
                    nc.vector.tensor_scalar_mul(t[:], ident[:], float(v))
                else:
                    nc.vector.memset(t[:], 0.0)
                tiles.append(t)
            return tiles

        def plain_evict(out16, negs=(None, None), fp32_out=None,
                        extra=None):
            """out16 = ((re_h, re_l), (im_h, im_l)) fp16 pairs, or
            fp32_out = (re32, im32) for the final U write."""
            def ev(p0, n, Ar, Br, Ai, Bi):
                for plane_i, (A, B) in enumerate(((Ar, Br), (Ai, Bi))):
                    t = combine(A, B)
                    if fp32_out is not None:
                        nc.sync.dma_start(osl(fp32_out[plane_i], p0, n),
                                          t[:])
                    else:
                        split_out(t, out16[plane_i], p0, n,
                                  neg_planes=negs[plane_i])
                if extra is not None:
                    extra(p0, n)
            return ev

        def chunk_evict(out16, cI, cM, cM2, Mp, M2p, diag_tiles,
                        fp32_out=None):
            """Horner eviction: out = psum + (cI*I + cM*M + cM2*M2)."""
            def ev(p0, n, Ar, Br, Ai, Bi):
                for plane_i, (A, B) in enumerate(((Ar, Br), (Ai, Bi))):
                    t = combine(A, B)
                    if plane_i == 0:
                        terms = [(Mp, cM.real), ("mi", -cM.imag),
                                 (M2p, cM2.real), ("m2i", -cM2.imag)]
                        terms = [(Mp[0], Mp[1], cM.real),
                                 (Mp[2], Mp[3], -cM.imag),
                                 (M2p[0], M2p[1], cM2.real),
                                 (M2p[2], M2p[3], -cM2.imag)]
                        dcoef = cI.real
                    else:
                        terms = [(Mp[0], Mp[1], cM.imag),
                                 (Mp[2], Mp[3], cM.real),
                                 (M2p[0], M2p[1], cM2.imag),
                                 (M2p[2], M2p[3], cM2.real)]
                        dcoef = cI.imag
                    terms = [tt for tt in terms if tt[2] != 0.0]
                    t = add_terms(t, p0, n, terms)
                    if dcoef != 0.0:
                        add_diag(t, p0, n, diag_tiles[plane_i])
                    if fp32_out is not None:
                        nc.sync.dma_start(osl(fp32_out[plane_i], p0, n),
                                          t[:])
                    else:
                        split_out(t, out16[plane_i], p0, n)
            return ev

        def transpose_pass(V4, T6):
            """T = V^T on fp16 planes (lossless): T6 = (tr_h, tr_l, ti_h,
            ti_l, tn_h, tn_l)."""
            plans = [(V4[0], ((T6[0], False),)),
                     (V4[1], ((T6[1], False),)),
                     (V4[2], ((T6[2], False), (T6[4], True))),
                     (V4[3], ((T6[3], False), (T6[5], True)))]
            pidx = 0
            for src, dsts in plans:
                S = stage_plane(src, "xrh")
                for bo in range(NB):
                    for bi in range(NB):
                        pt = ps.tile([P, P], f16, tag=f"pb{pidx % 8}",
                                     name=nm("pt"))
                        pidx += 1
                        nc.tensor.transpose(
                            pt[:], S[:, bi * D + bo * P: bi * D + bo * P + P],
                            ident16[:])
                        for (dstp, neg) in dsts:
                            st = est.tile([P, P], f16, tag="tev",
                                          name=nm("tt"))
                            if neg:
                                nc.vector.tensor_scalar_mul(st[:], pt[:],
                                                            -1.0)
                            else:
                                nc.vector.tensor_copy(st[:], pt[:])
                            nc.sync.dma_start(
                                dstp[bo * P:(bo + 1) * P,
                                     bi * P:(bi + 1) * P], st[:])

        def emit_b3_tile(p0, n, B3, Mp, M2p, diag_tiles):
            """B3 = c9*I + c10*M + c11*M2 on the DVE (riding M3's cadence)."""
            c9, c10, c11 = _COEF[9], _COEF[10], _COEF[11]
            for plane_i in range(2):
                if plane_i == 0:
                    terms = [(Mp[0], Mp[1], c10.real),
                             (Mp[2], Mp[3], -c10.imag),
                             (M2p[0], M2p[1], c11.real),
                             (M2p[2], M2p[3], -c11.imag)]
                    dcoef = c9.real
                else:
                    terms = [(Mp[0], Mp[1], c10.imag),
                             (Mp[2], Mp[3], c10.real),
                             (M2p[0], M2p[1], c11.imag),
                             (M2p[2], M2p[3], c11.real)]
                    dcoef = c9.imag
                terms = [tt for tt in terms if tt[2] != 0.0]
                z = est.tile([P, CB], f32, tag="ev", name=nm("bz"))
                nc.vector.memset(z[:], 0.0)
                t = add_terms(z, p0, n, terms)
                if dcoef != 0.0:
                    add_diag(t, p0, n, diag_tiles[plane_i])
                split_out(t, B3[plane_i], p0, n)

        # ---------------- per-gate flow ----------------
        for j, s in enumerate(slot_s):
            mm = m_in[j]
            # lhsT for M-products: L = conj(M) -> Lr = Mr, Li = -Mi (mn),
            # Lin = Mi
            L_M = (mm["mrh"], mm["mrl"], mm["mnh"], mm["mnl"],
                   mm["mih"], mm["mil"])
            X_M = (mm["mrh"], mm["mrl"], mm["mih"], mm["mil"])
            Mp = (mm["mrh"], mm["mrl"], mm["mih"], mm["mil"])

            M2 = [dplane(t) for t in ("m2rh", "m2rl", "m2ih", "m2il")]
            M3 = [dplane(t) for t in ("m3rh", "m3rl", "m3ih", "m3il",
                                      "m3nh", "m3nl")]
            B3 = [[dplane("b3rh"), dplane("b3rl")],
                  [dplane("b3ih"), dplane("b3il")]]
            QA = [dplane(t) for t in ("qarh", "qarl", "qaih", "qail")]
            QB = [dplane(t) for t in ("qbrh", "qbrl", "qbih", "qbil")]
            T6 = [dplane(t) for t in ("trh", "trl", "tih", "til",
                                      "tnh", "tnl")]

            # M2 = M @ M
            matmul_c(L_M, X_M,
                     plain_evict(((M2[0], M2[1]), (M2[2], M2[3]))))

            # M3 = M @ M2 with negated-imag planes; B3 rides along
            dg3 = make_diag_tiles(_COEF[9], "dg3_")
            M2p = (M2[0], M2[1], M2[2], M2[3])

            def b3_extra(p0, n):
                emit_b3_tile(p0, n, B3, Mp, M2p, dg3)

            matmul_c(L_M, (M2[0], M2[1], M2[2], M2[3]),
                     plain_evict(((M3[0], M3[1]), (M3[2], M3[3])),
                                 negs=(None, (M3[4], M3[5])),
                                 extra=b3_extra))

            L_M3 = (M3[0], M3[1], M3[4], M3[5], M3[2], M3[3])

            # Horner: Q = B3; for jc in (2,1,0): Q = M3 @ Q + B_jc
            prev = (B3[0][0], B3[0][1], B3[1][0], B3[1][1])
            for t_i, jc in enumerate((2, 1, 0)):
                last = (t_i == 2 and s == 0)
                tgt = QA if t_i % 2 == 0 else QB
                dg = make_diag_tiles(_COEF[3 * jc], f"dgh{t_i}_")
                matmul_c(L_M3, prev,
                         chunk_evict(((tgt[0], tgt[1]), (tgt[2], tgt[3])),
                                     _COEF[3 * jc], _COEF[3 * jc + 1],
                                     _COEF[3 * jc + 2], Mp, M2p, dg,
                                     fp32_out=(u_out[j] if last else None)))
                prev = tuple(tgt)

            # squarings: V <- V @ V, s times; last lands in u_out[j]
            V = prev
            other = QB if prev[0] is QA[0] else QA
            for t in range(s):
                transpose_pass(V, T6)
                L_T = (T6[0], T6[1], T6[2], T6[3], T6[4], T6[5])
                lastq = (t == s - 1)
                matmul_c(L_T, V,
                         plain_evict(((other[0], other[1]),
                                      (other[2], other[3])),
                                     fp32_out=(u_out[j] if lastq else None)))
                if not lastq:
                    V, other = tuple(other), list(V)

    nc.compile()
    return nc


def _get_program(slot_s):
    key = tuple(slot_s)
    if key not in _prog_cache:
        _prog_cache[key] = _build_program(key)
    return _prog_cache[key]


def _plan(th):
    a = np.abs(th) * LAM_BOUND
    order = np.argsort(-a)          # rank -> gate index
    slot_s = []
    for j in range(GPC):
        grp = a[order[j * NCORES:(j + 1) * NCORES]]
        s = max(0, math.ceil(math.log2(max(float(grp.max()), 1e-9) / X0)))
        slot_s.append(int(s))
    return order, slot_s


def _split16(x32):
    h = x32.astype(np.float16)
    l = ((x32 - h.astype(np.float32)) * np.float32(LOSC)).astype(np.float16)
    return h, l


def kernel(feature, theta, gens_re, gens_im):
    feature = np.asarray(feature)
    th = np.asarray(theta)[:, 0].astype(np.float64)
    gens_re = np.asarray(gens_re)
    gens_im = np.asarray(gens_im)

    order, slot_s = _plan(th)
    nc = _get_program(tuple(slot_s))

    ident = np.eye(P, dtype=np.float32)
    in_maps = []
    for c in range(NCORES):
        m = {"ident": ident}
        for j in range(GPC):
            k = int(order[j * NCORES + c])
            cc = np.float32(0.5 * th[k] / (2.0 ** slot_s[j]))
            r = gens_re[k].astype(np.float32)
            im = gens_im[k].astype(np.float32)
            Mr = cc * (r + r.T)
            Mi = cc * (im - im.T)
            mrh, mrl = _split16(Mr)
            mih, mil = _split16(Mi)
            m[f"mrh{j}"] = np.ascontiguousarray(mrh)
            m[f"mrl{j}"] = np.ascontiguousarray(mrl)
            m[f"mih{j}"] = np.ascontiguousarray(mih)
            m[f"mil{j}"] = np.ascontiguousarray(mil)
            m[f"mnh{j}"] = np.ascontiguousarray(-mih)
            m[f"mnl{j}"] = np.ascontiguousarray(-mil)
        in_maps.append(m)

    from concourse.bass_utils import run_bass_kernel_spmd
    res = run_bass_kernel_spmd(nc, in_maps, core_ids=list(range(NCORES)),
                               trace=TRACE)
    global LAST_RESULT
    LAST_RESULT = res

    U = {}
    for c in range(NCORES):
        for j in range(GPC):
            k = int(order[j * NCORES + c])
            U[k] = (res.results[c][f"u{j}re"].astype(np.float64)
                    + 1j * res.results[c][f"u{j}im"].astype(np.float64))

    psi = np.zeros(D, np.complex128)
    psi[:feature.shape[0]] = feature.astype(np.float64)
    psi /= np.linalg.norm(psi)
    for k in range(NK):
        psi = U[k] @ psi
    return (np.abs(psi) ** 2).astype(np.float32)
